# revision 1
# baseline (speedup 1.0000x reference)
"""Trainium2 Bass kernel for nn_GumbelPromptPool.

Reference computation (per batch row b):
    query  = mean_s x_embed[b]                       # [D]
    sim    = cos_sim(query, prompt_key)              # [P]
    4 rounds: idx_i = argmax(sim + gumbel_i);  sim[idx_i] -= 1000
    out[b] = concat(prompt[idx_0], ..., prompt[idx_3])   # [4*L, D]

The straight-through estimator weight w = soft + (hard - soft) is numerically
exactly the one-hot `hard` in fp32 (verified bit-exact against the jax
reference), so the output is purely gathered prompt rows; only the argmax
decisions matter.

Sharding: data-parallel over batch. 8 cores, 32 batch rows each;
prompt / prompt_key replicated; no collectives.
"""

import os
import sys

import numpy as np

for _p in ("/opt/trn_rl_repo",):
    if _p not in sys.path and os.path.isdir(_p):
        sys.path.append(_p)

import concourse.bass as bass
import concourse.mybir as mybir
import concourse.tile as tile
from concourse import bacc
from concourse.bass import IndirectOffsetOnAxis
from concourse.bass_utils import run_bass_kernel_spmd
from concourse.masks import make_identity

F32 = mybir.dt.float32
AF = mybir.ActivationFunctionType
ALU = mybir.AluOpType

N_CORES = 8
B, S, D = 256, 196, 1024
P, L, TOPK = 512, 8, 4
B_LOC = B // N_CORES          # 32
ROWS = B_LOC * S              # 6272 = 49 * 128
NBLK = ROWS // 128            # 49
XB = 5                        # x row-blocks per DMA tile
EPS_NORM = 1e-12
EPS_G = 1e-10
NEG = -1000.0


def _emit(tc):
    nc = tc.nc
    x = nc.dram_tensor("x", [B_LOC, S, D], F32, kind="ExternalInput").ap()
    pk = nc.dram_tensor("pk", [P, D], F32, kind="ExternalInput").ap()
    g = nc.dram_tensor("g", [TOPK, B_LOC, P], F32, kind="ExternalInput").ap()
    prompt = nc.dram_tensor("prompt", [P, L, D], F32, kind="ExternalInput").ap()
    w = nc.dram_tensor("w", [128, NBLK, B_LOC], F32, kind="ExternalInput").ap()
    out = nc.dram_tensor("out", [B_LOC, TOPK * L, D], F32, kind="ExternalOutput").ap()

    import contextlib
    ctx = contextlib.ExitStack()
    with ctx:
        consts = ctx.enter_context(tc.tile_pool(name="consts", bufs=1))
        xpool = ctx.enter_context(tc.tile_pool(name="xpool", bufs=2))
        scratch = ctx.enter_context(tc.tile_pool(name="scratch", bufs=2))
        rpool = ctx.enter_context(tc.tile_pool(name="rpool", bufs=2))
        gpool = ctx.enter_context(tc.tile_pool(name="gpool", bufs=2))
        psum = ctx.enter_context(tc.tile_pool(name="psum", bufs=1, space="PSUM"))
        psum2 = ctx.enter_context(tc.tile_pool(name="psum2", bufs=2, space="PSUM"))

        # ---- constants ----
        ident = consts.tile([128, 128], F32)
        make_identity(nc, ident)
        w_sb = consts.tile([128, NBLK, B_LOC], F32)
        nc.sync.dma_start(out=w_sb[:], in_=w[:])
        iota_i = consts.tile([B_LOC, P], mybir.dt.int32)
        nc.gpsimd.iota(iota_i[:], pattern=[[1, P]], base=0, channel_multiplier=0)
        iota_f = consts.tile([B_LOC, P], F32)
        nc.vector.tensor_copy(out=iota_f[:], in_=iota_i[:])
        g_sb = consts.tile([B_LOC, TOPK, P], F32)
        nc.sync.dma_start(out=g_sb[:], in_=g.rearrange("k b p -> b k p"))

        # ---- prompt_key: normalize rows, transpose to [D, P] ----
        key_sb = consts.tile([128, 4, D], F32)
        ksq = consts.tile([128, 4], F32)
        ksc = consts.tile([128, 4], F32)
        for pc in range(4):
            nc.sync.dma_start(out=key_sb[:, pc, :], in_=pk[128 * pc:128 * (pc + 1), :])
            sq = scratch.tile([128, D], F32, tag="sq128")
            nc.scalar.activation(out=sq[:], in_=key_sb[:, pc, :],
                                 func=AF.Square, accum_out=ksq[:, pc:pc + 1])
            nc.vector.tensor_scalar_max(ksc[:, pc:pc + 1], ksq[:, pc:pc + 1], EPS_NORM)
            nc.scalar.sqrt(ksc[:, pc:pc + 1], ksc[:, pc:pc + 1])
            nc.vector.reciprocal(out=ksc[:, pc:pc + 1], in_=ksc[:, pc:pc + 1])
            nc.vector.tensor_scalar_mul(key_sb[:, pc, :], key_sb[:, pc, :], ksc[:, pc:pc + 1])
        kT = consts.tile([128, 8, P], F32)
        for dc in range(8):
            pt = psum2.tile([128, P], F32, tag="pkt")
            for pc in range(4):
                nc.tensor.transpose(
                    out=pt[:, 128 * pc:128 * (pc + 1)],
                    in_=key_sb[:, pc, 128 * dc:128 * (dc + 1)],
                    identity=ident[:],
                )
            nc.vector.tensor_copy(out=kT[:, dc, :], in_=pt[:])

        # ---- mean over S via selector-matmul, accumulated in PSUM ----
        x_rows = x.rearrange("b s d -> (b s) d")
        psq = psum.tile([B_LOC, D], F32, tag="pq")
        for g0 in range(0, NBLK, XB):
            nb = min(XB, NBLK - g0)
            xt = xpool.tile([128, XB, D], F32, tag="xt")
            nc.sync.dma_start(
                out=xt[:, :nb, :],
                in_=x_rows[128 * g0:128 * (g0 + nb), :].rearrange(
                    "(n p) d -> p n d", p=128),
            )
            for j in range(nb):
                blk = g0 + j
                for nck in range(2):
                    nc.tensor.matmul(
                        out=psq[:, 512 * nck:512 * (nck + 1)],
                        lhsT=w_sb[:, blk, :],
                        rhs=xt[:, j, 512 * nck:512 * (nck + 1)],
                        start=(blk == 0),
                        stop=(blk == NBLK - 1),
                    )
        q_sb = consts.tile([B_LOC, D], F32)
        nc.vector.tensor_scalar_mul(q_sb[:], psq[:], 1.0 / float(S))

        # ---- query norm ----
        qsc = consts.tile([B_LOC, 1], F32)
        sq2 = scratch.tile([B_LOC, D], F32, tag="sq32")
        nc.scalar.activation(out=sq2[:], in_=q_sb[:],
                             func=AF.Square, accum_out=qsc[:])
        nc.vector.tensor_scalar_max(qsc[:], qsc[:], EPS_NORM)
        nc.scalar.sqrt(qsc[:], qsc[:])
        nc.vector.reciprocal(out=qsc[:], in_=qsc[:])

        # ---- transpose q to [D, B_LOC] ----
        qT = consts.tile([128, 8, B_LOC], F32)
        for dc in range(8):
            pq = psum2.tile([128, B_LOC], F32, tag="pqt")
            nc.tensor.transpose(
                out=pq[:],
                in_=q_sb[:, 128 * dc:128 * (dc + 1)],
                identity=ident[:B_LOC, :B_LOC],
            )
            nc.vector.tensor_copy(out=qT[:, dc, :], in_=pq[:])

        # ---- sim = (q/|q|) . key_n^T ----
        ps = psum.tile([B_LOC, P], F32, tag="psim")
        for dc in range(8):
            nc.tensor.matmul(
                out=ps[:], lhsT=qT[:, dc, :], rhs=kT[:, dc, :],
                start=(dc == 0), stop=(dc == 7),
            )
        simv = consts.tile([B_LOC, P], F32)
        nc.vector.tensor_scalar_mul(simv[:], ps[:], qsc[:, 0:1])

        # ---- 4 gumbel argmax rounds + gather ----
        prompt_flat = prompt.rearrange("p l d -> p (l d)")
        out_k = out.rearrange("b (k l) d -> b k (l d)", k=TOPK)
        for i in range(TOPK):
            v = rpool.tile([B_LOC, P], F32, tag="v")
            nc.vector.tensor_add(v[:], simv[:], g_sb[:, i, :])
            mx = rpool.tile([B_LOC, 8], F32, tag="mx")
            nc.vector.max(mx[:], v[:])
            idx = rpool.tile([B_LOC, 8], mybir.dt.uint32, tag="idx")
            nc.vector.max_index(idx[:], mx[:], v[:])
            if i < TOPK - 1:
                idxf = rpool.tile([B_LOC, 1], F32, tag="idxf")
                nc.vector.tensor_copy(out=idxf[:], in_=idx[:, 0:1])
                pen = rpool.tile([B_LOC, P], F32, tag="pen")
                nc.vector.tensor_scalar(
                    out=pen[:], in0=iota_f[:],
                    scalar1=idxf[:, 0:1], scalar2=NEG,
                    op0=ALU.is_equal, op1=ALU.mult,
                )
                nc.vector.tensor_add(simv[:], simv[:], pen[:])
            gt = gpool.tile([B_LOC, L * D], F32, tag="gath")
            nc.gpsimd.indirect_dma_start(
                out=gt[:],
                out_offset=None,
                in_=prompt_flat[:],
                in_offset=IndirectOffsetOnAxis(ap=idx[:, 0:1], axis=0),
            )
            nc.sync.dma_start(out=out_k[:, i, :], in_=gt[:])


def build_nc():
    nc = bacc.Bacc("TRN2", target_bir_lowering=False, debug=False,
                   num_devices=N_CORES)
    with tile.TileContext(nc) as tc:
        _emit(tc)
    nc.compile()
    return nc


def _build_w():
    wf = np.zeros((ROWS, B_LOC), dtype=np.float32)
    wf[np.arange(ROWS), np.arange(ROWS) // S] = 1.0
    return np.ascontiguousarray(
        wf.reshape(NBLK, 128, B_LOC).transpose(1, 0, 2))


_NC_CACHE = {}


def _get_nc():
    if "nc" not in _NC_CACHE:
        _NC_CACHE["nc"] = build_nc()
    return _NC_CACHE["nc"]


def make_in_maps(x_embed, prompt, prompt_key, gumbel_u):
    eps = np.float32(EPS_G)
    gn = -np.log(-np.log(gumbel_u.astype(np.float32) + eps) + eps)
    wm = _build_w()
    in_maps = []
    for c in range(N_CORES):
        bs = slice(c * B_LOC, (c + 1) * B_LOC)
        in_maps.append({
            "x": np.ascontiguousarray(x_embed[bs]),
            "pk": prompt_key,
            "g": np.ascontiguousarray(gn[:, bs]),
            "prompt": prompt,
            "w": wm,
        })
    return in_maps


def run(x_embed, prompt, prompt_key, gumbel_u, trace=False, tmpdir=None):
    nc = _get_nc()
    in_maps = make_in_maps(x_embed, prompt, prompt_key, gumbel_u)
    res = run_bass_kernel_spmd(nc, in_maps, list(range(N_CORES)),
                               trace=trace, tmpdir=tmpdir)
    full = np.concatenate([res.results[c]["out"] for c in range(N_CORES)], axis=0)
    return full, res


def kernel(x_embed, prompt, prompt_key, gumbel_u):
    full, _ = run(x_embed, prompt, prompt_key, gumbel_u, trace=False)
    return full



# revision 4
# speedup vs baseline: 1.0425x; 1.0425x over previous
"""Trainium2 Bass kernel for nn_GumbelPromptPool.

Reference computation (per batch row b):
    query  = mean_s x_embed[b]                       # [D]
    sim    = cos_sim(query, prompt_key)              # [P]
    4 rounds: idx_i = argmax(sim + gumbel_i);  sim[idx_i] -= 1000
    out[b] = concat(prompt[idx_0], ..., prompt[idx_3])   # [4*L, D]

The straight-through weight w = soft + (hard - soft) is numerically the
one-hot `hard` in fp32, so the output is purely gathered prompt rows.

Sharding: data-parallel over batch. 8 cores, 32 batch rows each;
prompt / prompt_key replicated; no collectives.

Layout: x is streamed with partitions = (b, s // 49), so each partition
line is one contiguous 196KB block of DRAM and the mean over S reduces to
DVE adds over the free axis (hidden under the DMA), plus one tiny
selector matmul at the end. The 1/S division is skipped — it cancels in
the cosine normalization.
"""

import os
import sys

import numpy as np

for _p in ("/opt/trn_rl_repo",):
    if _p not in sys.path and os.path.isdir(_p):
        sys.path.append(_p)

import concourse.bass as bass
import concourse.mybir as mybir
import concourse.tile as tile
from concourse import bacc
from concourse.bass import IndirectOffsetOnAxis
from concourse.bass_utils import run_bass_kernel_spmd
from concourse.masks import make_identity

F32 = mybir.dt.float32
AF = mybir.ActivationFunctionType
ALU = mybir.AluOpType

N_CORES = 8
B, S, D = 256, 196, 1024
P, L, TOPK = 512, 8, 4
B_LOC = B // N_CORES          # 32
G = 4                         # s-groups per batch -> partition = b*4 + g
SO = S // G                   # 49 slices per partition line
NSO = 7                       # slices per x tile
NT = SO // NSO                # 7 tiles
EPS_NORM = 1e-12
EPS_G = 1e-10
NEG = -1000.0


def _emit(tc):
    nc = tc.nc
    x = nc.dram_tensor("x", [B_LOC, S, D], F32, kind="ExternalInput").ap()
    pk = nc.dram_tensor("pk", [P, D], F32, kind="ExternalInput").ap()
    g = nc.dram_tensor("g", [TOPK, B_LOC, P], F32, kind="ExternalInput").ap()
    prompt = nc.dram_tensor("prompt", [P, L, D], F32, kind="ExternalInput").ap()
    w4 = nc.dram_tensor("w4", [128, B_LOC], F32, kind="ExternalInput").ap()
    out = nc.dram_tensor("out", [B_LOC, TOPK * L, D], F32, kind="ExternalOutput").ap()

    import contextlib
    ctx = contextlib.ExitStack()
    with ctx:
        consts = ctx.enter_context(tc.tile_pool(name="consts", bufs=1))
        xpool = ctx.enter_context(tc.tile_pool(name="xpool", bufs=2))
        rpool = ctx.enter_context(tc.tile_pool(name="rpool", bufs=2))
        gpool = ctx.enter_context(tc.tile_pool(name="gpool", bufs=2))
        psumT = ctx.enter_context(tc.tile_pool(name="psumT", bufs=2, space="PSUM"))
        psumS = ctx.enter_context(tc.tile_pool(name="psumS", bufs=1, space="PSUM"))

        # ---- x streaming: partition = (b, s//49), free = (so, d) ----
        # per-partition per-tile read is NSO*4KB contiguous DRAM.
        x_t = x.rearrange("b (g so) d -> (b g) so d", g=G)

        xt0 = xpool.tile([128, NSO, D], F32, tag="xt")
        nc.sync.dma_start(out=xt0[:], in_=x_t[:, 0:NSO, :])

        # ---- small inputs ride alongside ----
        key_sb = consts.tile([128, 4, D], F32)
        nc.sync.dma_start(out=key_sb[:], in_=pk.rearrange("(c p) d -> p c d", p=128))
        g_sb = consts.tile([B_LOC, TOPK, P], F32)
        nc.sync.dma_start(out=g_sb[:], in_=g.rearrange("k b p -> b k p"))
        w4_sb = consts.tile([128, B_LOC], F32)
        nc.sync.dma_start(out=w4_sb[:], in_=w4[:])

        # ---- constants ----
        iota_i = consts.tile([B_LOC, P], mybir.dt.int32)
        nc.gpsimd.iota(iota_i[:], pattern=[[1, P]], base=0, channel_multiplier=0)
        iota_f = consts.tile([B_LOC, P], F32)
        nc.gpsimd.tensor_copy(out=iota_f[:], in_=iota_i[:])
        ident = consts.tile([128, 128], F32)
        make_identity(nc, ident)

        # ---- prompt_key row norms (scalar engine) ----
        ksq = consts.tile([128, 4], F32)
        ksc = consts.tile([128, 4], F32)
        sq = consts.tile([128, D], F32)  # dead output for Square
        for c in range(4):
            nc.scalar.activation(out=sq[:], in_=key_sb[:, c, :],
                                 func=AF.Square, accum_out=ksq[:, c:c + 1])
        nc.gpsimd.tensor_scalar_max(ksc[:], ksq[:], EPS_NORM)
        nc.scalar.sqrt(ksc[:], ksc[:])

        acc = consts.tile([128, D], F32)
        kT = consts.tile([128, 8, P], F32)

        # ---- main loop: stream x, accumulate mean partials on DVE ----
        for t in range(NT):
            if t > 0:
                xt = xpool.tile([128, NSO, D], F32, tag="xt")
                nc.sync.dma_start(out=xt[:], in_=x_t[:, t * NSO:(t + 1) * NSO, :])
            else:
                xt = xt0
            for j in range(NSO):
                if t == 0 and j == 0:
                    continue
                elif t == 0 and j == 1:
                    nc.vector.tensor_add(acc[:], xt[:, 0, :], xt[:, 1, :])
                else:
                    nc.vector.tensor_add(acc[:], acc[:], xt[:, j, :])
            if t == 0:
                # key-norm finish + normalize + transpose, hidden under tile 1+
                nc.vector.reciprocal(out=ksc[:], in_=ksc[:])
                for c in range(4):
                    nc.scalar.activation(out=key_sb[:, c, :], in_=key_sb[:, c, :],
                                         func=AF.Copy, scale=ksc[:, c:c + 1])
                for dc in range(8):
                    pt = psumT.tile([128, P], F32, tag="pkt")
                    for c in range(4):
                        nc.tensor.transpose(
                            out=pt[:, 128 * c:128 * (c + 1)],
                            in_=key_sb[:, c, 128 * dc:128 * (dc + 1)],
                            identity=ident[:],
                        )
                    nc.scalar.activation(out=kT[:, dc, :], in_=pt[:], func=AF.Copy)

        # ---- stage 2: q_raw[b,d] = sum over the 4 partition groups ----
        psq = psumS.tile([B_LOC, D], F32, tag="pq")
        for nck in range(2):
            nc.tensor.matmul(
                out=psq[:, 512 * nck:512 * (nck + 1)],
                lhsT=w4_sb[:],
                rhs=acc[:, 512 * nck:512 * (nck + 1)],
                start=True, stop=True,
            )

        # ---- query norm (unscaled; 1/S cancels in cosine) ----
        qsc = consts.tile([B_LOC, 1], F32)
        sq2 = consts.tile([B_LOC, D], F32)
        nc.scalar.activation(out=sq2[:], in_=psq[:],
                             func=AF.Square, accum_out=qsc[:])
        nc.gpsimd.tensor_scalar_max(qsc[:], qsc[:], EPS_NORM)
        nc.scalar.sqrt(qsc[:], qsc[:])
        nc.vector.reciprocal(out=qsc[:], in_=qsc[:])

        # ---- transpose q to [D, B_LOC] ----
        q_sb = consts.tile([B_LOC, D], F32)
        nc.vector.tensor_copy(out=q_sb[:], in_=psq[:])
        qTp = psumS.tile([128, 8 * B_LOC], F32, tag="pqt")
        for dc in range(8):
            nc.tensor.transpose(
                out=qTp[:, B_LOC * dc:B_LOC * (dc + 1)],
                in_=q_sb[:, 128 * dc:128 * (dc + 1)],
                identity=ident[:B_LOC, :B_LOC],
            )
        qT_sb = consts.tile([128, 8 * B_LOC], F32)
        nc.scalar.activation(out=qT_sb[:], in_=qTp[:], func=AF.Copy)

        # ---- sim = (q/|q|) . key_n^T ----
        ps = psumS.tile([B_LOC, P], F32, tag="psim")
        for dc in range(8):
            nc.tensor.matmul(
                out=ps[:], lhsT=qT_sb[:, B_LOC * dc:B_LOC * (dc + 1)],
                rhs=kT[:, dc, :],
                start=(dc == 0), stop=(dc == 7),
            )
        simv = consts.tile([B_LOC, P], F32)
        nc.vector.tensor_scalar_mul(simv[:], ps[:], qsc[:, 0:1])

        # ---- 4 gumbel argmax rounds + gather ----
        prompt_flat = prompt.rearrange("p l d -> p (l d)")
        out_k = out.rearrange("b (k l) d -> b k (l d)", k=TOPK)
        for i in range(TOPK):
            v = rpool.tile([B_LOC, P], F32, tag="v")
            nc.vector.tensor_add(v[:], simv[:], g_sb[:, i, :])
            mx = rpool.tile([B_LOC, 8], F32, tag="mx")
            nc.vector.max(mx[:], v[:])
            idx = rpool.tile([B_LOC, 8], mybir.dt.uint32, tag="idx")
            nc.vector.max_index(idx[:], mx[:], v[:])
            if i < TOPK - 1:
                idxf = rpool.tile([B_LOC, 1], F32, tag="idxf")
                nc.vector.tensor_copy(out=idxf[:], in_=idx[:, 0:1])
                pen = rpool.tile([B_LOC, P], F32, tag="pen")
                nc.vector.tensor_scalar(
                    out=pen[:], in0=iota_f[:],
                    scalar1=idxf[:, 0:1], scalar2=NEG,
                    op0=ALU.is_equal, op1=ALU.mult,
                )
                nc.vector.tensor_add(simv[:], simv[:], pen[:])
            gt = gpool.tile([B_LOC, L * D], F32, tag="gath")
            nc.gpsimd.indirect_dma_start(
                out=gt[:],
                out_offset=None,
                in_=prompt_flat[:],
                in_offset=IndirectOffsetOnAxis(ap=idx[:, 0:1], axis=0),
            )
            nc.sync.dma_start(out=out_k[:, i, :], in_=gt[:])


def build_nc():
    nc = bacc.Bacc("TRN2", target_bir_lowering=False, debug=False,
                   num_devices=N_CORES)
    with tile.TileContext(nc) as tc:
        _emit(tc)
    nc.compile()
    return nc


def _build_w4():
    w = np.zeros((128, B_LOC), dtype=np.float32)
    w[np.arange(128), np.arange(128) // G] = 1.0
    return w


_NC_CACHE = {}


def _get_nc():
    if "nc" not in _NC_CACHE:
        _NC_CACHE["nc"] = build_nc()
    return _NC_CACHE["nc"]


def make_in_maps(x_embed, prompt, prompt_key, gumbel_u):
    eps = np.float32(EPS_G)
    gn = -np.log(-np.log(gumbel_u.astype(np.float32) + eps) + eps)
    wm = _build_w4()
    in_maps = []
    for c in range(N_CORES):
        bs = slice(c * B_LOC, (c + 1) * B_LOC)
        in_maps.append({
            "x": np.ascontiguousarray(x_embed[bs]),
            "pk": prompt_key,
            "g": np.ascontiguousarray(gn[:, bs]),
            "prompt": prompt,
            "w4": wm,
        })
    return in_maps


def run(x_embed, prompt, prompt_key, gumbel_u, trace=False, tmpdir=None):
    nc = _get_nc()
    in_maps = make_in_maps(x_embed, prompt, prompt_key, gumbel_u)
    res = run_bass_kernel_spmd(nc, in_maps, list(range(N_CORES)),
                               trace=trace, tmpdir=tmpdir)
    full = np.concatenate([res.results[c]["out"] for c in range(N_CORES)], axis=0)
    return full, res


def kernel(x_embed, prompt, prompt_key, gumbel_u):
    full, _ = run(x_embed, prompt, prompt_key, gumbel_u, trace=False)
    return full


# revision 10
# speedup vs baseline: 1.1992x; 1.1503x over previous
"""Trainium2 Bass kernel for nn_GumbelPromptPool.

Reference computation (per batch row b):
    query  = mean_s x_embed[b]                       # [D]
    sim    = cos_sim(query, prompt_key)              # [P]
    4 rounds: idx_i = argmax(sim + gumbel_i);  sim[idx_i] -= 1000
    out[b] = concat(prompt[idx_0], ..., prompt[idx_3])   # [4*L, D]

The straight-through weight w = soft + (hard - soft) is numerically the
one-hot `hard` in fp32, so the output is purely gathered prompt rows.

Sharding: data-parallel over batch. 8 cores, 32 batch rows each;
prompt / prompt_key replicated; no collectives.

Layout: x is streamed with partitions = (b, s // 49), so each partition
line is one contiguous block of DRAM. Per tile, slices are pre-reduced
on the DVE (hidden under the DMA) and folded into a PSUM query
accumulator via a tiny selector matmul on the otherwise-idle PE, so
stage 2 is off the critical tail. The 1/S division is skipped — it
cancels in the cosine normalization. Output gathers are split into
halves (and writes into quarters) to keep read+write DMA pipelined.
"""

import os
import sys

import numpy as np

for _p in ("/opt/trn_rl_repo",):
    if _p not in sys.path and os.path.isdir(_p):
        sys.path.append(_p)

import concourse.bass as bass
import concourse.mybir as mybir
import concourse.tile as tile
from concourse import bacc
from concourse.bass import IndirectOffsetOnAxis
from concourse.bass_utils import run_bass_kernel_spmd
from concourse.masks import make_identity

F32 = mybir.dt.float32
AF = mybir.ActivationFunctionType
ALU = mybir.AluOpType

N_CORES = 8
B, S, D = 256, 196, 1024
P, L, TOPK = 512, 8, 4
B_LOC = B // N_CORES          # 32
G = 4                         # s-groups per batch -> partition = b*4 + g
SO = S // G                   # 49 slices per partition line
TILES = [8, 8, 8, 8, 8, 8, 1]
EPS_NORM = 1e-12
EPS_G = 1e-10
NEG = -1000.0
HLD = (L * D) // 2            # gather half width
QLD = (L * D) // 4            # write quarter width


def _emit(tc):
    nc = tc.nc
    x = nc.dram_tensor("x", [B_LOC, S, D], F32, kind="ExternalInput").ap()
    pk = nc.dram_tensor("pk", [P, D], F32, kind="ExternalInput").ap()
    g = nc.dram_tensor("g", [TOPK, B_LOC, P], F32, kind="ExternalInput").ap()
    prompt = nc.dram_tensor("prompt", [P, L, D], F32, kind="ExternalInput").ap()
    w4 = nc.dram_tensor("w4", [128, B_LOC], F32, kind="ExternalInput").ap()
    out = nc.dram_tensor("out", [B_LOC, TOPK * L, D], F32, kind="ExternalOutput").ap()

    import contextlib
    ctx = contextlib.ExitStack()
    with ctx:
        consts = ctx.enter_context(tc.tile_pool(name="consts", bufs=1))
        xpool = ctx.enter_context(tc.tile_pool(name="xpool", bufs=2))
        apool = ctx.enter_context(tc.tile_pool(name="apool", bufs=2))
        rpool = ctx.enter_context(tc.tile_pool(name="rpool", bufs=2))
        gpool = ctx.enter_context(tc.tile_pool(name="gpool", bufs=2))
        psumT = ctx.enter_context(tc.tile_pool(name="psumT", bufs=2, space="PSUM"))
        psumS = ctx.enter_context(tc.tile_pool(name="psumS", bufs=1, space="PSUM"))

        # ---- x streaming: partition = (b, s//49), free = (so, d) ----
        x_t = x.rearrange("b (g so) d -> (b g) so d", g=G)

        # first two x tiles issue before the small inputs
        so0 = [sum(TILES[:t]) for t in range(len(TILES))]
        xts = []
        for t in range(2):
            xt = xpool.tile([128, TILES[0], D], F32, tag="xt")
            nc.sync.dma_start(
                out=xt[:, 0:TILES[t], :],
                in_=x_t[:, so0[t]:so0[t] + TILES[t], :])
            xts.append(xt)

        key_sb = consts.tile([128, 4, D], F32)
        nc.sync.dma_start(out=key_sb[:], in_=pk.rearrange("(c p) d -> p c d", p=128))
        g_sb = consts.tile([B_LOC, TOPK, P], F32)
        nc.sync.dma_start(out=g_sb[:], in_=g.rearrange("k b p -> b k p"))
        w4_sb = consts.tile([128, B_LOC], F32)
        nc.sync.dma_start(out=w4_sb[:], in_=w4[:])

        # ---- constants ----
        iota_i = consts.tile([B_LOC, P], mybir.dt.int32)
        nc.gpsimd.iota(iota_i[:], pattern=[[1, P]], base=0, channel_multiplier=0)
        iota_f = consts.tile([B_LOC, P], F32)
        nc.gpsimd.tensor_copy(out=iota_f[:], in_=iota_i[:])
        ident = consts.tile([128, 128], F32)
        make_identity(nc, ident)

        # ---- prompt_key row norms (scalar engine) ----
        ksq = consts.tile([128, 4], F32)
        ksc = consts.tile([128, 4], F32)
        sq = consts.tile([128, D], F32)  # dead output for Square
        for c in range(4):
            nc.scalar.activation(out=sq[:], in_=key_sb[:, c, :],
                                 func=AF.Square, accum_out=ksq[:, c:c + 1])
        nc.gpsimd.tensor_scalar_max(ksc[:], ksq[:], EPS_NORM)
        nc.scalar.sqrt(ksc[:], ksc[:])

        kT = consts.tile([128, 8, P], F32)
        psq = psumS.tile([B_LOC, D], F32, tag="pq")

        # ---- main loop: stream x; DVE pre-reduce per tile; PE folds into psq ----
        for t, nso in enumerate(TILES):
            if t >= 2:
                xt = xpool.tile([128, TILES[0], D], F32, tag="xt")
                nc.sync.dma_start(out=xt[:, 0:nso, :],
                                  in_=x_t[:, so0[t]:so0[t] + nso, :])
            else:
                xt = xts[t]
            if nso > 1:
                acc = apool.tile([128, D], F32, tag="acc")
                nc.vector.tensor_add(acc[:], xt[:, 0, :], xt[:, 1, :])
                for j in range(2, nso):
                    nc.vector.tensor_add(acc[:], acc[:], xt[:, j, :])
                rhs_src = acc
                rhs = lambda lo, hi: rhs_src[:, lo:hi]
            else:
                rhs_src = xt
                rhs = lambda lo, hi: rhs_src[:, 0, lo:hi]
            for nck in range(2):
                nc.tensor.matmul(
                    out=psq[:, 512 * nck:512 * (nck + 1)],
                    lhsT=w4_sb[:],
                    rhs=rhs(512 * nck, 512 * (nck + 1)),
                    start=(t == 0), stop=(t == len(TILES) - 1),
                )
            if t == 0:
                # key-norm finish + normalize + transpose, hidden under tiles 1+
                nc.vector.reciprocal(out=ksc[:], in_=ksc[:])
                for c in range(4):
                    nc.scalar.activation(out=key_sb[:, c, :], in_=key_sb[:, c, :],
                                         func=AF.Copy, scale=ksc[:, c:c + 1])
                for dc in range(8):
                    pt = psumT.tile([128, P], F32, tag="pkt")
                    for c in range(4):
                        nc.tensor.transpose(
                            out=pt[:, 128 * c:128 * (c + 1)],
                            in_=key_sb[:, c, 128 * dc:128 * (dc + 1)],
                            identity=ident[:],
                        )
                    nc.scalar.activation(out=kT[:, dc, :], in_=pt[:], func=AF.Copy)

        # ---- query norm (unscaled; 1/S cancels in cosine) ----
        qsc = consts.tile([B_LOC, 1], F32)
        sq2 = consts.tile([B_LOC, D], F32)
        nc.scalar.activation(out=sq2[:], in_=psq[:],
                             func=AF.Square, accum_out=qsc[:])
        nc.gpsimd.tensor_scalar_max(qsc[:], qsc[:], EPS_NORM)
        nc.scalar.sqrt(qsc[:], qsc[:])
        nc.vector.reciprocal(out=qsc[:], in_=qsc[:])

        # ---- transpose q to [D, B_LOC] ----
        q_sb = consts.tile([B_LOC, D], F32)
        nc.vector.tensor_copy(out=q_sb[:], in_=psq[:])
        qTp = psumS.tile([128, 8 * B_LOC], F32, tag="pqt")
        for dc in range(8):
            nc.tensor.transpose(
                out=qTp[:, B_LOC * dc:B_LOC * (dc + 1)],
                in_=q_sb[:, 128 * dc:128 * (dc + 1)],
                identity=ident[:B_LOC, :B_LOC],
            )
        qT_sb = consts.tile([128, 8 * B_LOC], F32)
        nc.scalar.activation(out=qT_sb[:], in_=qTp[:], func=AF.Copy)

        # ---- sim = (q/|q|) . key_n^T ----
        ps = psumS.tile([B_LOC, P], F32, tag="psim")
        for dc in range(8):
            nc.tensor.matmul(
                out=ps[:], lhsT=qT_sb[:, B_LOC * dc:B_LOC * (dc + 1)],
                rhs=kT[:, dc, :],
                start=(dc == 0), stop=(dc == 7),
            )
        simv = consts.tile([B_LOC, P], F32)
        nc.vector.tensor_scalar_mul(simv[:], ps[:], qsc[:, 0:1])

        # ---- 4 gumbel argmax rounds + split gathers / writes ----
        # prompt viewed as half-rows [2P, HLD]; gather row 2*idx (+HLD offset
        # for the second half) — indirect DMA scales idx by the AP row size.
        prompt_h = prompt.rearrange("p (h l2) d -> (p h) (l2 d)", h=2)
        out_k = out.rearrange("b (k l) d -> b k (l d)", k=TOPK)
        for i in range(TOPK):
            v = rpool.tile([B_LOC, P], F32, tag="v")
            nc.vector.tensor_add(v[:], simv[:], g_sb[:, i, :])
            mx = rpool.tile([B_LOC, 8], F32, tag="mx")
            nc.vector.max(mx[:], v[:])
            idx = rpool.tile([B_LOC, 8], mybir.dt.uint32, tag="idx")
            nc.vector.max_index(idx[:], mx[:], v[:])
            idx2 = rpool.tile([B_LOC, 1], mybir.dt.uint32, tag="idx2")
            nc.vector.tensor_scalar_mul(idx2[:], idx[:, 0:1], 2)
            if i < TOPK - 1:
                idxf = rpool.tile([B_LOC, 1], F32, tag="idxf")
                nc.vector.tensor_copy(out=idxf[:], in_=idx[:, 0:1])
                pen = rpool.tile([B_LOC, P], F32, tag="pen")
                nc.vector.tensor_scalar(
                    out=pen[:], in0=iota_f[:],
                    scalar1=idxf[:, 0:1], scalar2=NEG,
                    op0=ALU.is_equal, op1=ALU.mult,
                )
                nc.vector.tensor_add(simv[:], simv[:], pen[:])
            for h in range(2):
                gt = gpool.tile([B_LOC, HLD], F32, tag=f"g{h}")
                nc.gpsimd.indirect_dma_start(
                    out=gt[:],
                    out_offset=None,
                    in_=prompt_h[:],
                    in_offset=IndirectOffsetOnAxis(ap=idx2[:, 0:1], axis=0),
                    element_offset=h * HLD,
                )
                for c in range(2):
                    lo = c * QLD
                    nc.sync.dma_start(
                        out=out_k[:, i, h * HLD + lo:h * HLD + lo + QLD],
                        in_=gt[:, lo:lo + QLD])


def build_nc():
    nc = bacc.Bacc("TRN2", target_bir_lowering=False, debug=False,
                   num_devices=N_CORES)
    with tile.TileContext(nc) as tc:
        _emit(tc)
    nc.compile()
    return nc


def _build_w4():
    w = np.zeros((128, B_LOC), dtype=np.float32)
    w[np.arange(128), np.arange(128) // G] = 1.0
    return w


_NC_CACHE = {}


def _get_nc():
    if "nc" not in _NC_CACHE:
        _NC_CACHE["nc"] = build_nc()
    return _NC_CACHE["nc"]


def make_in_maps(x_embed, prompt, prompt_key, gumbel_u):
    eps = np.float32(EPS_G)
    gn = -np.log(-np.log(gumbel_u.astype(np.float32) + eps) + eps)
    wm = _build_w4()
    in_maps = []
    for c in range(N_CORES):
        bs = slice(c * B_LOC, (c + 1) * B_LOC)
        in_maps.append({
            "x": np.ascontiguousarray(x_embed[bs]),
            "pk": prompt_key,
            "g": np.ascontiguousarray(gn[:, bs]),
            "prompt": prompt,
            "w4": wm,
        })
    return in_maps


def run(x_embed, prompt, prompt_key, gumbel_u, trace=False, tmpdir=None):
    nc = _get_nc()
    in_maps = make_in_maps(x_embed, prompt, prompt_key, gumbel_u)
    res = run_bass_kernel_spmd(nc, in_maps, list(range(N_CORES)),
                               trace=trace, tmpdir=tmpdir)
    full = np.concatenate([res.results[c]["out"] for c in range(N_CORES)], axis=0)
    return full, res


def kernel(x_embed, prompt, prompt_key, gumbel_u):
    full, _ = run(x_embed, prompt, prompt_key, gumbel_u, trace=False)
    return full
